# revision 31
# baseline (speedup 1.0000x reference)
"""Masked dot-product attention on 8 Trainium2 NeuronCores.

Strategy (per core): head-parallel sharding. B*H = 64 (batch, head) pairs are
split 8 per core; each core runs the full attention for its heads.

Per-head pipeline (S=2048, DK=64), all in "S-transposed" layout so the PV
matmul needs no transpose of the huge exp matrix:
  S_T[kj, qi] = K @ Q^T        (PE, bf16, psum [128 kj, 1024 qi] strips)
  E_T = exp(S_T / sqrt(dk))    (ScalarE, PSUM -> SBUF bf16; no max-shift:
                                logits are ~N(0,1), exp can't overflow, and
                                masked entries underflow to exactly 0)
  E_T *= maskT (0/1 bf16)      (DVE, 2x mode, in-place)
  O_T[dv', qi] += V'[kj]^T E_T (PE accumulate over kj; V' has a ones column
                                so row dv'=64 accumulates the softmax denom)
  O = (O_T^T)[:, :64] * recip(O_T^T[:, 64])   (PE transpose + DVE)

The int32 [S, S] mask is shared by all heads: converted once per core to a
transposed 0/1 bf16 copy held in SBUF (keep = 1 - mask).
"""

import math

import numpy as np

import concourse.bass as bass
import concourse.mybir as mybir
import concourse.tile as tile
from concourse import bacc
from concourse.masks import make_identity

F32 = mybir.dt.float32
BF16 = mybir.dt.bfloat16
I32 = mybir.dt.int32
AF = mybir.ActivationFunctionType
ALU = mybir.AluOpType

N_CORES = 8


def build_attention_nc(nheads: int, S: int, DK: int, scale: float) -> bass.Bass:
    nc = bacc.Bacc("TRN2", target_bir_lowering=False, debug=False,
                   num_devices=N_CORES)

    q_d = nc.dram_tensor("queries", [nheads, S, DK], F32, kind="ExternalInput")
    k_d = nc.dram_tensor("keys", [nheads, S, DK], F32, kind="ExternalInput")
    v_d = nc.dram_tensor("values", [nheads, S, DK], F32, kind="ExternalInput")
    m_d = nc.dram_tensor("mask", [S, S], I32, kind="ExternalInput")
    o_d = nc.dram_tensor("out", [nheads, S, DK], F32, kind="ExternalOutput")

    DV1 = DK + 1          # V plus a ones column for softmax denominators
    n_kj = S // 128       # kj tiles per head
    QBLK = min(512, S)    # qi span of one O_T accumulator
    n_qblk = S // QBLK
    OC = QBLK // 128      # 128-row output chunks per block
    CH = S // 128         # 128-row chunks along seq

    with tile.TileContext(nc) as tc:
        with (
            tc.tile_pool(name="consts", bufs=1) as consts,
            tc.tile_pool(name="maskT", bufs=1) as maskpool,
            tc.tile_pool(name="mstage", bufs=2) as mstage,
            tc.tile_pool(name="stage", bufs=2) as stage,
            tc.tile_pool(name="qkT", bufs=2) as qkt,
            tc.tile_pool(name="vp", bufs=2) as vp,
            tc.tile_pool(name="ep", bufs=17) as ep,
            tc.tile_pool(name="outp", bufs=4) as outp,
            tc.tile_pool(name="small", bufs=4) as small,
            tc.tile_pool(name="spsum", bufs=2, space="PSUM") as spsum,
            tc.tile_pool(name="opsum", bufs=2, space="PSUM") as opsum,
            tc.tile_pool(name="tpsum", bufs=2, space="PSUM") as tpsum,
            tc.tile_pool(name="dram_scr", bufs=2, space="DRAM") as dram_scr,
        ):
            ident_f = consts.tile([DV1, DV1], F32)
            make_identity(nc, ident_f)
            ident_f128 = consts.tile([128, 128], F32)
            make_identity(nc, ident_f128)
            ident_b128 = consts.tile([128, 128], BF16)
            make_identity(nc, ident_b128)

            maskT = [
                maskpool.tile([128, S], BF16, tag=f"maskT{kt}",
                              name=f"maskT_{kt}")
                for kt in range(n_kj)
            ]

            def emit_mask_strip(kt):
                # maskT[kt][j, qi] = 1 - mask[qi, kt*128 + j]
                m_strip = mstage.tile([128, CH, 128], I32, tag="mraw",
                                      name=f"mraw_{kt}")
                m_src = m_d[:, kt * 128 : (kt + 1) * 128].rearrange(
                    "(c p) j -> p c j", p=128
                )
                h2 = CH // 2
                nc.sync.dma_start(out=m_strip[:, :h2, :], in_=m_src[:, :h2, :])
                nc.gpsimd.dma_start(
                    out=m_strip[:, h2:, :], in_=m_src[:, h2:, :]
                )
                mb = mstage.tile([128, CH, 128], BF16, tag="mbf",
                                 name=f"mbf_{kt}")
                conv_eng = nc.vector
                conv_eng.tensor_scalar(
                    out=mb, in0=m_strip, scalar1=-1.0, scalar2=1.0,
                    op0=ALU.mult, op1=ALU.add,
                )
                for g in range(0, CH, 8):
                    gn = min(8, CH - g)
                    ps = tpsum.tile([128, 8, 128], BF16, tag="t",
                                    name=f"mtp_{kt}_{g}")
                    for c in range(gn):
                        nc.tensor.transpose(ps[:, c, :], mb[:, g + c, :],
                                            ident_b128)
                    nc.vector.tensor_copy(
                        maskT[kt][:, g * 128 : (g + gn) * 128], ps[:, :gn, :]
                    )

            assert nheads % 2 == 0
            for hp in range(nheads // 2):
                # ---- Q^T, K^T. Pair 0: PE identity-matmul transposes
                # (short latency; PE idle at startup). Later pairs: DRAM-
                # roundtrip DMA transpose, prefetched off-PE during steady
                # state. Head i lands on partitions 64i..64i+63 either way.
                qT_kT = []
                for name, src in (("q", q_d), ("k", k_d)):
                    tT = qkt.tile([128, S], BF16, tag=f"{name}T",
                                  name=f"{name}T_{hp}")
                    if hp == 0:
                        nat2 = stage.tile([128, CH, 2, DK], F32, tag="natq",
                                          name=f"nat2_{name}_{hp}")
                        for i in (0, 1):
                            nc.sync.dma_start(
                                out=nat2[:, :, i, :],
                                in_=src[2 * hp + i].rearrange(
                                    "(c p) d -> p c d", p=128
                                ),
                            )
                        for g in range(0, CH, 4):
                            gn = min(4, CH - g)
                            ps = tpsum.tile([128, 4, 128], F32, tag="t",
                                            name=f"tstg_{name}_{hp}_{g}")
                            for c in range(gn):
                                for i in (0, 1):
                                    nc.tensor.matmul(
                                        ps[64 * i : 64 * i + 64, c, :],
                                        nat2[:, g + c, i, :],
                                        ident_f128,
                                        start=True, stop=True,
                                        tile_position=(0, 64 * i),
                                    )
                            nc.vector.tensor_copy(
                                tT[:, g * 128 : (g + gn) * 128], ps[:, :gn, :]
                            )
                    else:
                        natb = stage.tile([128, CH, 2, DK], BF16, tag="natb",
                                          name=f"natb_{name}_{hp}")
                        for i in (0, 1):
                            nat = stage.tile([128, CH, DK], F32, tag="natq",
                                             name=f"nat_{name}_{hp}_{i}")
                            nc.sync.dma_start(
                                out=nat,
                                in_=src[2 * hp + i].rearrange(
                                    "(c p) d -> p c d", p=128
                                ),
                            )
                            nc.vector.tensor_copy(natb[:, :, i, :], nat)
                        scr = dram_scr.tile([S, 2 * DK], BF16, tag="scr",
                                            name=f"scr_{name}_{hp}")
                        nc.sync.dma_start(
                            out=scr.rearrange("(c p) e -> p c e", p=128),
                            in_=natb.rearrange("p c i d -> p c (i d)"),
                        )
                        nc.sync.dma_start(out=tT, in_=scr, transpose=True)
                    qT_kT.append(tT)
                qT2, kT2 = qT_kT

                v_nat2 = stage.tile([128, CH, 2, DK], F32, tag="nat",
                                    name=f"v_nat_{hp}")
                v_eng = nc.scalar if hp == 0 else nc.sync
                for i in (0, 1):
                    v_eng.dma_start(
                        out=v_nat2[:, :, i, :],
                        in_=v_d[2 * hp + i].rearrange(
                            "(c p) d -> p c d", p=128
                        ),
                    )
                v1s = []
                for i in (0, 1):
                    v1 = vp.tile([128, CH, DV1], BF16, tag=f"v1_{i}",
                                 name=f"v1_{2 * hp + i}")
                    nc.vector.tensor_copy(v1[:, :, 0:DK], v_nat2[:, :, i, :])
                    nc.gpsimd.memset(v1[:, :, DK:DV1], 1.0)
                    v1s.append(v1)

                # ---- main loop: block outer, kj inner; the pair shares each
                # psum strip [h0 blk | h1 blk]: the two K=64 QK^T matmuls use
                # different PE row groups (concurrent) and ACT/DVE handle
                # both heads per instruction. exp needs no mask, so ACT runs
                # ahead of mask-strip arrival through the deep E pool.
                def emit_step(qb, kj, ps_o):
                    q0 = qb * QBLK
                    ps_s = spsum.tile([128, 2 * QBLK], F32, tag="s",
                                      name=f"ps_s_{hp}_{qb}_{kj}")
                    for i in (0, 1):
                        nc.tensor.matmul(
                            ps_s[:, i * QBLK : (i + 1) * QBLK],
                            kT2[64 * i : 64 * i + DK,
                                kj * 128 : (kj + 1) * 128],
                            qT2[64 * i : 64 * i + DK, q0 : q0 + QBLK],
                            start=True, stop=True,
                        )
                    e_t = ep.tile([128, 2 * QBLK], BF16, tag="e",
                                  name=f"e_{hp}_{qb}_{kj}")
                    nc.scalar.activation(e_t, ps_s, AF.Exp, scale=scale)
                    msl = maskT[kj][:, q0 : q0 + QBLK]
                    mdup = bass.AP(
                        tensor=msl.tensor, offset=msl.offset,
                        ap=[msl.ap[0], [0, 2], msl.ap[-1]],
                    )
                    nc.vector.tensor_mul(e_t, e_t, mdup)
                    for i in (0, 1):
                        nc.tensor.matmul(
                            ps_o[i],
                            v1s[i][:, kj, :],
                            e_t[:, i * QBLK : (i + 1) * QBLK],
                            start=(kj == 0), stop=(kj == n_kj - 1),
                            skip_group_check=True,
                        )

                def emit_output(qb, ps_o):
                    q0 = qb * QBLK
                    for i in (0, 1):
                        h = 2 * hp + i
                        ot_sb = outp.tile([DV1, QBLK], F32, tag="ot",
                                          name=f"ot_{h}_{qb}")
                        nc.vector.tensor_copy(ot_sb, ps_o[i])
                        ps_nat = tpsum.tile([128, OC, DV1], F32, tag="t",
                                            name=f"ps_nat_{h}_{qb}")
                        for c in range(OC):
                            nc.tensor.transpose(
                                ps_nat[:, c, :],
                                ot_sb[:, c * 128 : (c + 1) * 128],
                                ident_f,
                            )
                        rec = small.tile([128, OC], F32, tag="rec",
                                         name=f"rec_{h}_{qb}")
                        nc.vector.reciprocal(rec, ps_nat[:, :, DK])
                        o_sb = outp.tile([128, OC, DK], F32, tag="osb",
                                         name=f"o_sb_{h}_{qb}")
                        rb = bass.AP(tensor=rec.tensor, offset=rec.offset,
                                     ap=[rec.ap[0], rec.ap[-1], [0, DK]])
                        nc.vector.tensor_mul(o_sb, ps_nat[:, :, 0:DK], rb)
                        nc.sync.dma_start(
                            out=o_d[h, q0 : q0 + QBLK, :].rearrange(
                                "(c p) d -> p c d", p=128
                            ),
                            in_=o_sb,
                        )

                for qb in range(n_qblk):
                    ps_o = [
                        opsum.tile([DV1, QBLK], F32, tag="o",
                                   name=f"ps_o_{hp}_{qb}_{i}")
                        for i in (0, 1)
                    ]
                    if hp == 0 and qb == 0:
                        for kt0 in range(min(3, n_kj)):
                            emit_mask_strip(kt0)
                    for kj in range(n_kj):
                        emit_step(qb, kj, ps_o)
                        if hp == 0 and qb == 0 and kj + 3 < n_kj:
                            emit_mask_strip(kj + 3)
                    emit_output(qb, ps_o)

    nc.compile()
    return nc


_NC_CACHE: dict = {}


def _get_nc(nheads, S, DK, scale):
    key = (nheads, S, DK, scale)
    if key not in _NC_CACHE:
        _NC_CACHE[key] = build_attention_nc(nheads, S, DK, scale)
    return _NC_CACHE[key]


def kernel(queries, keys, values, d_k, mask):
    from concourse.bass_utils import run_bass_kernel_spmd

    B, H, S, DK = queries.shape
    BH = B * H
    assert BH % N_CORES == 0
    hpc = BH // N_CORES
    scale = 1.0 / math.sqrt(float(d_k))

    nc = _get_nc(hpc, S, DK, scale)

    qf = np.ascontiguousarray(queries.reshape(BH, S, DK)).astype(np.float32)
    kf = np.ascontiguousarray(keys.reshape(BH, S, DK)).astype(np.float32)
    vf = np.ascontiguousarray(values.reshape(BH, S, DK)).astype(np.float32)
    mf = np.ascontiguousarray(mask.reshape(S, S)).astype(np.int32)

    in_maps = [
        {
            "queries": qf[c * hpc : (c + 1) * hpc],
            "keys": kf[c * hpc : (c + 1) * hpc],
            "values": vf[c * hpc : (c + 1) * hpc],
            "mask": mf,
        }
        for c in range(N_CORES)
    ]
    res = run_bass_kernel_spmd(nc, in_maps, core_ids=list(range(N_CORES)))
    out = np.concatenate([r["out"] for r in res.results], axis=0)
    return out.reshape(B, H, S, DK).astype(queries.dtype)


# revision 32
# speedup vs baseline: 1.0640x; 1.0640x over previous
"""Masked dot-product attention on 8 Trainium2 NeuronCores.

Strategy (per core): head-parallel sharding. B*H = 64 (batch, head) pairs are
split 8 per core; each core runs the full attention for its heads.

Per-head pipeline (S=2048, DK=64), all in "S-transposed" layout so the PV
matmul needs no transpose of the huge exp matrix:
  S_T[kj, qi] = K @ Q^T        (PE, bf16, psum [128 kj, 1024 qi] strips)
  E_T = exp(S_T / sqrt(dk))    (ScalarE, PSUM -> SBUF bf16; no max-shift:
                                logits are ~N(0,1), exp can't overflow, and
                                masked entries underflow to exactly 0)
  E_T *= maskT (0/1 bf16)      (DVE, 2x mode, in-place)
  O_T[dv', qi] += V'[kj]^T E_T (PE accumulate over kj; V' has a ones column
                                so row dv'=64 accumulates the softmax denom)
  O = (O_T^T)[:, :64] * recip(O_T^T[:, 64])   (PE transpose + DVE)

The int32 [S, S] mask is shared by all heads: converted once per core to a
transposed 0/1 bf16 copy held in SBUF (keep = 1 - mask).
"""

import math

import numpy as np

import concourse.bass as bass
import concourse.mybir as mybir
import concourse.tile as tile
from concourse import bacc
from concourse.masks import make_identity

F32 = mybir.dt.float32
BF16 = mybir.dt.bfloat16
I32 = mybir.dt.int32
AF = mybir.ActivationFunctionType
ALU = mybir.AluOpType

N_CORES = 8


def build_attention_nc(nheads: int, S: int, DK: int, scale: float) -> bass.Bass:
    nc = bacc.Bacc("TRN2", target_bir_lowering=False, debug=False,
                   num_devices=N_CORES)

    q_d = nc.dram_tensor("queries", [nheads, S, DK], F32, kind="ExternalInput")
    k_d = nc.dram_tensor("keys", [nheads, S, DK], F32, kind="ExternalInput")
    v_d = nc.dram_tensor("values", [nheads, S, DK], F32, kind="ExternalInput")
    m_d = nc.dram_tensor("mask", [S, S], I32, kind="ExternalInput")
    o_d = nc.dram_tensor("out", [nheads, S, DK], F32, kind="ExternalOutput")

    DV1 = DK + 1          # V plus a ones column for softmax denominators
    n_kj = S // 128       # kj tiles per head
    QBLK = min(512, S)    # qi span of one O_T accumulator
    n_qblk = S // QBLK
    OC = QBLK // 128      # 128-row output chunks per block
    CH = S // 128         # 128-row chunks along seq

    with tile.TileContext(nc) as tc:
        with (
            tc.tile_pool(name="consts", bufs=1) as consts,
            tc.tile_pool(name="maskT", bufs=1) as maskpool,
            tc.tile_pool(name="mstage", bufs=3) as mstage,
            tc.tile_pool(name="stage", bufs=2) as stage,
            tc.tile_pool(name="qkT", bufs=2) as qkt,
            tc.tile_pool(name="vp", bufs=2) as vp,
            tc.tile_pool(name="ep", bufs=20) as ep,
            tc.tile_pool(name="outp", bufs=4) as outp,
            tc.tile_pool(name="small", bufs=4) as small,
            tc.tile_pool(name="spsum", bufs=2, space="PSUM") as spsum,
            tc.tile_pool(name="opsum", bufs=2, space="PSUM") as opsum,
            tc.tile_pool(name="tpsum", bufs=2, space="PSUM") as tpsum,
            tc.tile_pool(name="dram_scr", bufs=2, space="DRAM") as dram_scr,
        ):
            ident_f = consts.tile([DV1, DV1], F32)
            make_identity(nc, ident_f)
            ident_f128 = consts.tile([128, 128], F32)
            make_identity(nc, ident_f128)
            ident_b128 = consts.tile([128, 128], BF16)
            make_identity(nc, ident_b128)

            maskT = [
                maskpool.tile([128, S], BF16, tag=f"maskT{kt}",
                              name=f"maskT_{kt}")
                for kt in range(n_kj)
            ]

            def emit_mask_strip(kt):
                # maskT[kt][j, qi] = 1 - mask[qi, kt*128 + j], processed in
                # qi-halves: the first two qi-blocks only read half 0 of each
                # strip (subtile deps), halving effective mask latency.
                m_src = m_d[:, kt * 128 : (kt + 1) * 128].rearrange(
                    "(c p) j -> p c j", p=128
                )
                hc = max(1, CH // 2)
                for hf in range(CH // hc):
                    c0, c1 = hf * hc, (hf + 1) * hc
                    m_half = mstage.tile([128, hc, 128], I32, tag="mraw",
                                         name=f"mraw_{kt}_{hf}")
                    eng = nc.sync if (kt + hf) % 2 == 0 else nc.gpsimd
                    eng.dma_start(out=m_half, in_=m_src[:, c0:c1, :])
                    mb = mstage.tile([128, hc, 128], BF16, tag="mbf",
                                     name=f"mbf_{kt}_{hf}")
                    nc.vector.tensor_scalar(
                        out=mb, in0=m_half, scalar1=-1.0, scalar2=1.0,
                        op0=ALU.mult, op1=ALU.add,
                    )
                    ps = tpsum.tile([128, 8, 128], BF16, tag="t",
                                    name=f"mtp_{kt}_{hf}")
                    for c in range(hc):
                        nc.tensor.transpose(ps[:, c, :], mb[:, c, :],
                                            ident_b128)
                    nc.vector.tensor_copy(
                        maskT[kt][:, c0 * 128 : c1 * 128], ps[:, :hc, :]
                    )

            assert nheads % 2 == 0
            for hp in range(nheads // 2):
                # ---- Q^T, K^T. Pair 0: PE identity-matmul transposes
                # (short latency; PE idle at startup). Later pairs: DRAM-
                # roundtrip DMA transpose, prefetched off-PE during steady
                # state. Head i lands on partitions 64i..64i+63 either way.
                qT_kT = []
                for name, src in (("q", q_d), ("k", k_d)):
                    tT = qkt.tile([128, S], BF16, tag=f"{name}T",
                                  name=f"{name}T_{hp}")
                    if hp == 0:
                        nat2 = stage.tile([128, CH, 2, DK], F32, tag="natq",
                                          name=f"nat2_{name}_{hp}")
                        for i in (0, 1):
                            nc.sync.dma_start(
                                out=nat2[:, :, i, :],
                                in_=src[2 * hp + i].rearrange(
                                    "(c p) d -> p c d", p=128
                                ),
                            )
                        for g in range(0, CH, 4):
                            gn = min(4, CH - g)
                            ps = tpsum.tile([128, 4, 128], F32, tag="t",
                                            name=f"tstg_{name}_{hp}_{g}")
                            for c in range(gn):
                                for i in (0, 1):
                                    nc.tensor.matmul(
                                        ps[64 * i : 64 * i + 64, c, :],
                                        nat2[:, g + c, i, :],
                                        ident_f128,
                                        start=True, stop=True,
                                        tile_position=(0, 64 * i),
                                    )
                            nc.vector.tensor_copy(
                                tT[:, g * 128 : (g + gn) * 128], ps[:, :gn, :]
                            )
                    else:
                        natb = stage.tile([128, CH, 2, DK], BF16, tag="natb",
                                          name=f"natb_{name}_{hp}")
                        for i in (0, 1):
                            nat = stage.tile([128, CH, DK], F32, tag="natq",
                                             name=f"nat_{name}_{hp}_{i}")
                            nc.sync.dma_start(
                                out=nat,
                                in_=src[2 * hp + i].rearrange(
                                    "(c p) d -> p c d", p=128
                                ),
                            )
                            nc.vector.tensor_copy(natb[:, :, i, :], nat)
                        scr = dram_scr.tile([S, 2 * DK], BF16, tag="scr",
                                            name=f"scr_{name}_{hp}")
                        nc.sync.dma_start(
                            out=scr.rearrange("(c p) e -> p c e", p=128),
                            in_=natb.rearrange("p c i d -> p c (i d)"),
                        )
                        nc.sync.dma_start(out=tT, in_=scr, transpose=True)
                    qT_kT.append(tT)
                qT2, kT2 = qT_kT

                v_nat2 = stage.tile([128, CH, 2, DK], F32, tag="nat",
                                    name=f"v_nat_{hp}")
                v_eng = nc.scalar if hp == 0 else nc.sync
                for i in (0, 1):
                    v_eng.dma_start(
                        out=v_nat2[:, :, i, :],
                        in_=v_d[2 * hp + i].rearrange(
                            "(c p) d -> p c d", p=128
                        ),
                    )
                v1s = []
                for i in (0, 1):
                    v1 = vp.tile([128, CH, DV1], BF16, tag=f"v1_{i}",
                                 name=f"v1_{2 * hp + i}")
                    nc.vector.tensor_copy(v1[:, :, 0:DK], v_nat2[:, :, i, :])
                    nc.gpsimd.memset(v1[:, :, DK:DV1], 1.0)
                    v1s.append(v1)

                # ---- main loop: block outer, kj inner; the pair shares each
                # psum strip [h0 blk | h1 blk]: the two K=64 QK^T matmuls use
                # different PE row groups (concurrent) and ACT/DVE handle
                # both heads per instruction. exp needs no mask, so ACT runs
                # ahead of mask-strip arrival through the deep E pool.
                def emit_step(qb, kj, ps_o):
                    q0 = qb * QBLK
                    ps_s = spsum.tile([128, 2 * QBLK], F32, tag="s",
                                      name=f"ps_s_{hp}_{qb}_{kj}")
                    for i in (0, 1):
                        nc.tensor.matmul(
                            ps_s[:, i * QBLK : (i + 1) * QBLK],
                            kT2[64 * i : 64 * i + DK,
                                kj * 128 : (kj + 1) * 128],
                            qT2[64 * i : 64 * i + DK, q0 : q0 + QBLK],
                            start=True, stop=True,
                        )
                    e_t = ep.tile([128, 2 * QBLK], BF16, tag="e",
                                  name=f"e_{hp}_{qb}_{kj}")
                    nc.scalar.activation(e_t, ps_s, AF.Exp, scale=scale)
                    msl = maskT[kj][:, q0 : q0 + QBLK]
                    mdup = bass.AP(
                        tensor=msl.tensor, offset=msl.offset,
                        ap=[msl.ap[0], [0, 2], msl.ap[-1]],
                    )
                    nc.vector.tensor_mul(e_t, e_t, mdup)
                    for i in (0, 1):
                        nc.tensor.matmul(
                            ps_o[i],
                            v1s[i][:, kj, :],
                            e_t[:, i * QBLK : (i + 1) * QBLK],
                            start=(kj == 0), stop=(kj == n_kj - 1),
                            skip_group_check=True,
                        )

                def emit_output(qb, ps_o):
                    q0 = qb * QBLK
                    for i in (0, 1):
                        h = 2 * hp + i
                        ot_sb = outp.tile([DV1, QBLK], F32, tag="ot",
                                          name=f"ot_{h}_{qb}")
                        nc.vector.tensor_copy(ot_sb, ps_o[i])
                        ps_nat = tpsum.tile([128, OC, DV1], F32, tag="t",
                                            name=f"ps_nat_{h}_{qb}")
                        for c in range(OC):
                            nc.tensor.transpose(
                                ps_nat[:, c, :],
                                ot_sb[:, c * 128 : (c + 1) * 128],
                                ident_f,
                            )
                        rec = small.tile([128, OC], F32, tag="rec",
                                         name=f"rec_{h}_{qb}")
                        nc.vector.reciprocal(rec, ps_nat[:, :, DK])
                        o_sb = outp.tile([128, OC, DK], F32, tag="osb",
                                         name=f"o_sb_{h}_{qb}")
                        rb = bass.AP(tensor=rec.tensor, offset=rec.offset,
                                     ap=[rec.ap[0], rec.ap[-1], [0, DK]])
                        nc.vector.tensor_mul(o_sb, ps_nat[:, :, 0:DK], rb)
                        nc.sync.dma_start(
                            out=o_d[h, q0 : q0 + QBLK, :].rearrange(
                                "(c p) d -> p c d", p=128
                            ),
                            in_=o_sb,
                        )

                for qb in range(n_qblk):
                    ps_o = [
                        opsum.tile([DV1, QBLK], F32, tag="o",
                                   name=f"ps_o_{hp}_{qb}_{i}")
                        for i in (0, 1)
                    ]
                    if hp == 0 and qb == 0:
                        for kt0 in range(min(3, n_kj)):
                            emit_mask_strip(kt0)
                    for kj in range(n_kj):
                        emit_step(qb, kj, ps_o)
                        if hp == 0 and qb == 0 and kj + 3 < n_kj:
                            emit_mask_strip(kj + 3)
                    emit_output(qb, ps_o)

    nc.compile()
    return nc


_NC_CACHE: dict = {}


def _get_nc(nheads, S, DK, scale):
    key = (nheads, S, DK, scale)
    if key not in _NC_CACHE:
        _NC_CACHE[key] = build_attention_nc(nheads, S, DK, scale)
    return _NC_CACHE[key]


def kernel(queries, keys, values, d_k, mask):
    from concourse.bass_utils import run_bass_kernel_spmd

    B, H, S, DK = queries.shape
    BH = B * H
    assert BH % N_CORES == 0
    hpc = BH // N_CORES
    scale = 1.0 / math.sqrt(float(d_k))

    nc = _get_nc(hpc, S, DK, scale)

    qf = np.ascontiguousarray(queries.reshape(BH, S, DK)).astype(np.float32)
    kf = np.ascontiguousarray(keys.reshape(BH, S, DK)).astype(np.float32)
    vf = np.ascontiguousarray(values.reshape(BH, S, DK)).astype(np.float32)
    mf = np.ascontiguousarray(mask.reshape(S, S)).astype(np.int32)

    in_maps = [
        {
            "queries": qf[c * hpc : (c + 1) * hpc],
            "keys": kf[c * hpc : (c + 1) * hpc],
            "values": vf[c * hpc : (c + 1) * hpc],
            "mask": mf,
        }
        for c in range(N_CORES)
    ]
    res = run_bass_kernel_spmd(nc, in_maps, core_ids=list(range(N_CORES)))
    out = np.concatenate([r["out"] for r in res.results], axis=0)
    return out.reshape(B, H, S, DK).astype(queries.dtype)
